# revision 4
# baseline (speedup 1.0000x reference)
"""Trainium2 Bass kernel for nn_CPCircuitLayer_63350767616542 (embedding_lookup).

Reference math:
    seq_emb = einsum("bsh,rh->bsr", hidden_states, W_seq)          # [B,S,R]
    hid_fac = hidden_embeddings * cp_weight[0][None, :]            # [H,R]
    out[b,n] = sum_r seq_emb[b, si[n], r] * hid_fac[hi[n], r]      # [B,N]
    return out.reshape(B, S, N // S)

all_indices is the row-major cartesian product of (seq_idx, hidden_idx), so the
gather is the identity and the whole layer collapses to a two-matmul chain:
    P = hidden_states @ W_seq.T @ hid_fac.T                        # [B,S,H]
A host-side fallback gather handles any non-cartesian index list.

Sharding: flatten (B,S) -> 2048 rows, shard rows across the 8 cores (256 rows
per core, data-parallel, no collectives). Each core computes
    TT = W_seq @ X_c.T                  ([64, 256], via lhsT = W_seq.T)
    O_c = X_c @ W_seq.T @ hid_fac.T     ([256, 512])
The host pre-packs per-core SBUF images (contraction dim on partitions, fully
contiguous DMA descriptors) and replicates the small rank-sized factors, then
concatenates the core outputs.

Device kernel (raw bass, hand-scheduled; matmuls in float32r, ~2e-4 rel err):
    SP:   input DMAs: [W image | xt k-chunks 0-1], [xt k-chunks 2-3], [hid_fac.T]
          then the two output DMAs
    Pool: memset of the PE warm-up tile
    PE:   warm-up dummy matmuls (lift the HAM clock gate during the DMA
          window), mm1 (4 accumulating matmuls, starting as soon as the first
          xt half lands), mm2 (one matmul per 128-row output chunk)
    DVE:  tt copy (PSUM->SBUF, split per output chunk), 2 output copies
"""

import os

import numpy as np

B, S, H, R = 2, 1024, 512, 64
N_CORES = 8
ROWS = B * S                      # 2048 flattened rows
RPC = ROWS // N_CORES             # 256 rows per core
KC = H // 128                     # 4 contraction chunks of 128
MC = RPC // 128                   # 2 output row chunks of 128
W_COLS = KC * R                   # 256 cols of the packed W image
HALF = (KC // 2) * RPC            # 512 xt cols per half-DMA

# matmul operand dtype: "f32" (exact, 4 cyc/row), "f32r" (~2e-4 rel err,
# full rate at N>=256), "bf16" (~3e-3 rel err, full rate + half DMA bytes)
MM_DTYPE = os.environ.get("BASS_MM_DTYPE", "f32r")
N_DUMMY = int(os.environ.get("BASS_N_DUMMY", "12"))

_cache = {}
LAST_RESULT = None                # BassKernelResults of the most recent run


def _np_in_dtype():
    if MM_DTYPE == "bf16":
        import ml_dtypes

        return ml_dtypes.bfloat16
    return np.float32


def _get_nc():
    key = ("nc", MM_DTYPE, N_DUMMY)
    if key in _cache:
        return _cache[key]

    import concourse.bass as bass
    import concourse.mybir as mybir

    f32 = mybir.dt.float32
    mmdt = {
        "f32": mybir.dt.float32,
        "f32r": mybir.dt.float32r,
        "bf16": mybir.dt.bfloat16,
    }[MM_DTYPE]

    nc = bass.Bass(
        "TRN2",
        target_bir_lowering=False,
        debug=False,
        num_devices=N_CORES,
    )

    xw_d = nc.dram_tensor("xw", [128, W_COLS + HALF], mmdt, kind="ExternalInput")
    xt23_d = nc.dram_tensor("xt23", [128, HALF], mmdt, kind="ExternalInput")
    h_d = nc.dram_tensor("h", [R, H], mmdt, kind="ExternalInput")
    out_d = nc.dram_tensor("out", [RPC, H], f32, kind="ExternalOutput")

    with (
        nc.sbuf_tensor([128, W_COLS + HALF], mmdt) as xw_sb,
        nc.sbuf_tensor([R, H], mmdt) as h_sb,
        nc.sbuf_tensor([128, HALF], mmdt) as xt23_sb,
        nc.sbuf_tensor([R, RPC], mmdt) as tt_sb,
        nc.sbuf_tensor([128, H], f32) as o0_sb,
        nc.sbuf_tensor([128, H], f32) as o1_sb,
        nc.sbuf_tensor([128, 256], mybir.dt.bfloat16) as dummy_sb,
        nc.psum_tensor([R, RPC], f32) as tt_ps,
        nc.psum_tensor([128, H], f32) as o0_ps,
        nc.psum_tensor([128, H], f32) as o1_ps,
        nc.psum_tensor([128, 256], f32) as dummy_ps,
        nc.semaphore("s_xt01") as s_xt01,
        nc.semaphore("s_xt23") as s_xt23,
        nc.semaphore("s_h") as s_h,
        nc.semaphore("s_init") as s_init,
        nc.semaphore("s_pe") as s_pe,
        nc.semaphore("s_dve") as s_dve,
        nc.semaphore("s_oc") as s_oc,
        nc.semaphore("s_out") as s_out,
        nc.Block(no_gpsimd_drain=True) as block,
    ):
        o_sb = [o0_sb, o1_sb]
        o_ps = [o0_ps, o1_ps]
        w_sb = xw_sb[:, 0:W_COLS]

        def xt_chunk(k):
            if k < 2:
                return xw_sb[:, W_COLS + k * RPC : W_COLS + (k + 1) * RPC]
            return xt23_sb[:, (k - 2) * RPC : (k - 1) * RPC]

        @block.sync
        def _(sync):
            sync.dma_start(xw_sb[:], xw_d.ap()).then_inc(s_xt01, 16)
            sync.dma_start(xt23_sb[:], xt23_d.ap()).then_inc(s_xt23, 16)
            sync.dma_start(h_sb[:], h_d.ap()).then_inc(s_h, 16)

        @block.gpsimd
        def _(gpsimd):
            gpsimd.memset(dummy_sb[:], 0.0).then_inc(s_init, 1)

        @block.tensor
        def _(tensor):
            tensor.wait_ge(s_init, 1)
            for _ in range(N_DUMMY):
                nc.tensor.matmul(
                    dummy_ps[:], dummy_sb[:, 0:128], dummy_sb[:], start=True,
                    stop=True,
                )
            tensor.wait_ge(s_xt01, 16)
            for k in range(2):
                nc.tensor.matmul(
                    tt_ps[:],
                    w_sb[:, k * R : (k + 1) * R],
                    xt_chunk(k),
                    start=(k == 0),
                    stop=False,
                )
            tensor.wait_ge(s_xt23, 16)
            for k in range(2, 4):
                mm = nc.tensor.matmul(
                    tt_ps[:],
                    w_sb[:, k * R : (k + 1) * R],
                    xt_chunk(k),
                    start=False,
                    stop=(k == 3),
                )
            mm.then_inc(s_pe, 1)
            tensor.wait_ge(s_h, 16)
            for m in range(MC):
                tensor.wait_ge(s_dve, m + 1)
                nc.tensor.matmul(
                    o_ps[m][:],
                    tt_sb[:, m * 128 : (m + 1) * 128],
                    h_sb[:],
                    start=True,
                    stop=True,
                ).then_inc(s_pe, 1)

        @block.vector
        def _(vector):
            vector.wait_ge(s_pe, 1)
            # per-m halves so mm2[m] starts after its own half lands
            for m in range(MC):
                nc.vector.tensor_copy(
                    tt_sb[:, m * 128 : (m + 1) * 128],
                    tt_ps[:, m * 128 : (m + 1) * 128],
                ).then_inc(s_dve, 1)
            for m in range(MC):
                vector.wait_ge(s_pe, 2 + m)
                nc.vector.tensor_copy(o_sb[m][:], o_ps[m][:]).then_inc(s_oc, 1)

        @block.sync
        def _(sync):
            for m in range(MC):
                sync.wait_ge(s_oc, m + 1)
                sync.dma_start(
                    out_d.ap()[m * 128 : (m + 1) * 128, :], o_sb[m][:]
                ).then_inc(s_out, 16)
            sync.wait_ge(s_out, 16 * MC)

    # Drop the unused const-AP memsets bass emits unconditionally in its
    # preamble (the BIR verifier itself flags them as having no reader);
    # they serialize ~380ns on Pool ahead of the startup barrier.
    b0 = nc.m.functions[0].blocks[0]
    b0.instructions = [
        i
        for i in b0.instructions
        if not (
            type(i).__name__ == "InstMemset"
            and str(getattr(i.outs[0], "memref", "")).startswith("const-")
        )
    ]

    _cache[key] = nc
    return nc


def _pack_inputs(hidden_states, W_seq, hidden_embeddings, cp_weight):
    """Build the per-core packed SBUF images.

    xt image:   xt[c][p, k*RPC + n] = X[c*RPC + n, k*128 + p]
    W image:    w[p, k*R + r]       = W_seq[r, k*128 + p]
    h image:    h[r, j]             = hid_fac[j, r] = (hidden_embeddings * cp)[j, r]
    """
    ind = _np_in_dtype()
    X = hidden_states.reshape(ROWS, H)
    xt = (
        X.astype(ind)
        .reshape(N_CORES, RPC, KC, 128)  # [c, n, k, p]
        .transpose(0, 3, 2, 1)           # [c, p, k, n]
        .reshape(N_CORES, 128, KC * RPC)
    )
    w = (
        W_seq.astype(np.float32)
        .reshape(R, KC, 128)             # [r, k, p]
        .transpose(2, 1, 0)              # [p, k, r]
        .reshape(128, W_COLS)
        .astype(ind)
    )
    xw = np.ascontiguousarray(
        np.concatenate(
            [np.broadcast_to(w, (N_CORES, 128, W_COLS)), xt[:, :, :HALF]], axis=2
        )
    )                                    # [c, 128, W_COLS + HALF]
    xt23 = np.ascontiguousarray(xt[:, :, HALF:])
    h = np.ascontiguousarray(
        (hidden_embeddings * cp_weight[0][None, :]).T.astype(ind)
    )                                    # [64, 512]
    return xw, xt23, h


def _run_device(xw, xt23, h, trace=False, **run_kwargs):
    global LAST_RESULT
    from concourse.bass_utils import run_bass_kernel_spmd

    nc = _get_nc()
    in_maps = [{"xw": xw[c], "xt23": xt23[c], "h": h} for c in range(N_CORES)]
    res = run_bass_kernel_spmd(
        nc, in_maps, core_ids=list(range(N_CORES)), trace=trace, **run_kwargs
    )
    LAST_RESULT = res
    return np.concatenate([r["out"] for r in res.results], axis=0)  # [2048, 512]


def _host_reference(hidden_states, W_seq, hidden_embeddings, cp_weight):
    """Pure-numpy fallback (correct, host-only)."""
    hid_fac = hidden_embeddings * cp_weight[0][None, :]
    X = hidden_states.reshape(ROWS, H)
    return (X @ W_seq.T @ hid_fac.T).astype(np.float32)


def kernel(hidden_states, all_indices, W_seq, hidden_embeddings, cp_weight,
           trace=False, **run_kwargs):
    hidden_states = np.asarray(hidden_states, dtype=np.float32)
    W_seq = np.asarray(W_seq, dtype=np.float32)
    hidden_embeddings = np.asarray(hidden_embeddings, dtype=np.float32)
    cp_weight = np.asarray(cp_weight, dtype=np.float32)
    all_indices = np.asarray(all_indices)

    try:
        xw, xt23, h = _pack_inputs(
            hidden_states, W_seq, hidden_embeddings, cp_weight
        )
        Y = _run_device(xw, xt23, h, trace=trace, **run_kwargs)
    except Exception as e:  # device unavailable/wedged: stay correct on host
        import traceback

        traceback.print_exc()
        print(f"kernel: device path failed ({type(e).__name__}); "
              "falling back to host compute")
        Y = _host_reference(hidden_states, W_seq, hidden_embeddings, cp_weight)

    P = Y.reshape(B, S, H)

    n = all_indices.shape[0]
    si = all_indices[:, 0].astype(np.int64)
    hi = all_indices[:, 1].astype(np.int64)
    flat = si * H + hi
    if n == S * H and np.array_equal(flat, np.arange(S * H, dtype=np.int64)):
        return P  # cartesian-product indices: the gather is the identity
    return P.reshape(B, S * H)[:, flat].reshape(B, S, n // S)


# revision 6
# speedup vs baseline: 1.0699x; 1.0699x over previous
"""Trainium2 Bass kernel for nn_CPCircuitLayer_63350767616542 (embedding_lookup).

Reference math:
    seq_emb = einsum("bsh,rh->bsr", hidden_states, W_seq)          # [B,S,R]
    hid_fac = hidden_embeddings * cp_weight[0][None, :]            # [H,R]
    out[b,n] = sum_r seq_emb[b, si[n], r] * hid_fac[hi[n], r]      # [B,N]
    return out.reshape(B, S, N // S)

all_indices is the row-major cartesian product of (seq_idx, hidden_idx), so the
gather is the identity and the whole layer collapses to a two-matmul chain:
    P = hidden_states @ W_seq.T @ hid_fac.T                        # [B,S,H]
A host-side fallback gather handles any non-cartesian index list.

Sharding: flatten (B,S) -> 2048 rows, shard rows across the 8 cores (256 rows
per core, data-parallel, no collectives). Each core computes
    TT = W_seq @ X_c.T                  ([64, 256], via lhsT = W_seq.T)
    O_c = X_c @ W_seq.T @ hid_fac.T     ([256, 512])
The host pre-packs per-core SBUF images (contraction dim on partitions, fully
contiguous DMA descriptors) and replicates the small rank-sized factors, then
concatenates the core outputs.

Device kernel (raw bass, hand-scheduled; matmuls in float32r, ~2e-4 rel err):
    SP:   input DMAs: [W image | xt k-chunks 0-1], [xt k-chunks 2-3], [hid_fac.T]
          then the two output DMAs
    Pool: memset of the PE warm-up tile
    PE:   warm-up dummy matmuls (lift the HAM clock gate during the DMA
          window), mm1 (4 accumulating matmuls, starting as soon as the first
          xt half lands), mm2 (one matmul per 128-row output chunk)
    DVE:  tt copy (PSUM->SBUF, split per output chunk), 2 output copies
"""

import os

import numpy as np

B, S, H, R = 2, 1024, 512, 64
N_CORES = 8
ROWS = B * S                      # 2048 flattened rows
RPC = ROWS // N_CORES             # 256 rows per core
KC = H // 128                     # 4 contraction chunks of 128
MC = RPC // 128                   # 2 output row chunks of 128
W_COLS = KC * R                   # 256 cols of the packed W image
HALF = (KC // 2) * RPC            # 512 xt cols per half-DMA

# matmul operand dtype: "f32" (exact, 4 cyc/row), "f32r" (~2e-4 rel err,
# full rate at N>=256), "bf16" (~3e-3 rel err, full rate + half DMA bytes)
MM_DTYPE = os.environ.get("BASS_MM_DTYPE", "f32r")
N_DUMMY = int(os.environ.get("BASS_N_DUMMY", "12"))

_cache = {}
LAST_RESULT = None                # BassKernelResults of the most recent run


def _np_in_dtype():
    if MM_DTYPE == "bf16":
        import ml_dtypes

        return ml_dtypes.bfloat16
    return np.float32


def _get_nc():
    key = ("nc", MM_DTYPE, N_DUMMY)
    if key in _cache:
        return _cache[key]

    import concourse.bass as bass
    import concourse.mybir as mybir

    f32 = mybir.dt.float32
    mmdt = {
        "f32": mybir.dt.float32,
        "f32r": mybir.dt.float32r,
        "bf16": mybir.dt.bfloat16,
    }[MM_DTYPE]

    nc = bass.Bass(
        "TRN2",
        target_bir_lowering=False,
        debug=False,
        num_devices=N_CORES,
    )

    xw_d = nc.dram_tensor("xw", [128, W_COLS + HALF], mmdt, kind="ExternalInput")
    xt23_d = nc.dram_tensor("xt23", [128, HALF], mmdt, kind="ExternalInput")
    h_d = nc.dram_tensor("h", [R, H], mmdt, kind="ExternalInput")
    out_d = nc.dram_tensor("out", [RPC, H], f32, kind="ExternalOutput")

    with (
        nc.sbuf_tensor([128, W_COLS + HALF], mmdt) as xw_sb,
        nc.sbuf_tensor([R, H], mmdt) as h_sb,
        nc.sbuf_tensor([128, HALF], mmdt) as xt23_sb,
        nc.sbuf_tensor([R, RPC], mmdt) as tt_sb,
        nc.sbuf_tensor([128, H], f32) as o0_sb,
        nc.sbuf_tensor([128, H], f32) as o1_sb,
        nc.sbuf_tensor([128, 256], mybir.dt.bfloat16) as dummy_sb,
        nc.psum_tensor([R, RPC], f32) as tt_ps,
        nc.psum_tensor([128, H], f32) as o0_ps,
        nc.psum_tensor([128, H], f32) as o1_ps,
        nc.psum_tensor([128, 256], f32) as dummy_ps,
        nc.semaphore("s_xt01") as s_xt01,
        nc.semaphore("s_xt23") as s_xt23,
        nc.semaphore("s_h") as s_h,
        nc.semaphore("s_init") as s_init,
        nc.semaphore("s_pe") as s_pe,
        nc.semaphore("s_dve") as s_dve,
        nc.semaphore("s_oc") as s_oc,
        nc.semaphore("s_out") as s_out,
        nc.Block(no_gpsimd_drain=True) as block,
    ):
        o_sb = [o0_sb, o1_sb]
        o_ps = [o0_ps, o1_ps]
        w_sb = xw_sb[:, 0:W_COLS]

        def xt_chunk(k):
            if k < 2:
                return xw_sb[:, W_COLS + k * RPC : W_COLS + (k + 1) * RPC]
            return xt23_sb[:, (k - 2) * RPC : (k - 1) * RPC]

        @block.sync
        def _(sync):
            sync.dma_start(xw_sb[:], xw_d.ap()).then_inc(s_xt01, 16)
            sync.dma_start(xt23_sb[:], xt23_d.ap()).then_inc(s_xt23, 16)
            sync.dma_start(h_sb[:], h_d.ap()).then_inc(s_h, 16)

        @block.gpsimd
        def _(gpsimd):
            gpsimd.memset(dummy_sb[:], 0.0).then_inc(s_init, 1)

        @block.tensor
        def _(tensor):
            tensor.wait_ge(s_init, 1)
            for _ in range(N_DUMMY):
                nc.tensor.matmul(
                    dummy_ps[:], dummy_sb[:, 0:128], dummy_sb[:], start=True,
                    stop=True,
                )
            tensor.wait_ge(s_xt01, 16)
            for k in range(2):
                nc.tensor.matmul(
                    tt_ps[:],
                    w_sb[:, k * R : (k + 1) * R],
                    xt_chunk(k),
                    start=(k == 0),
                    stop=False,
                )
            tensor.wait_ge(s_xt23, 16)
            for k in range(2, 4):
                mm = nc.tensor.matmul(
                    tt_ps[:],
                    w_sb[:, k * R : (k + 1) * R],
                    xt_chunk(k),
                    start=False,
                    stop=(k == 3),
                )
            mm.then_inc(s_pe, 1)
            tensor.wait_ge(s_h, 16)
            for m in range(MC):
                tensor.wait_ge(s_dve, m + 1)
                nc.tensor.matmul(
                    o_ps[m][:],
                    tt_sb[:, m * 128 : (m + 1) * 128],
                    h_sb[:],
                    start=True,
                    stop=True,
                ).then_inc(s_pe, 1)

        @block.vector
        def _(vector):
            vector.wait_ge(s_pe, 1)
            # per-m halves so mm2[m] starts after its own half lands
            for m in range(MC):
                nc.vector.tensor_copy(
                    tt_sb[:, m * 128 : (m + 1) * 128],
                    tt_ps[:, m * 128 : (m + 1) * 128],
                ).then_inc(s_dve, 1)
            for m in range(MC):
                vector.wait_ge(s_pe, 2 + m)
                nc.vector.tensor_copy(o_sb[m][:], o_ps[m][:]).then_inc(s_oc, 1)

        @block.sync
        def _(sync):
            for m in range(MC):
                sync.wait_ge(s_oc, m + 1)
                sync.dma_start(
                    out_d.ap()[m * 128 : (m + 1) * 128, :], o_sb[m][:]
                ).then_inc(s_out, 16)
            sync.wait_ge(s_out, 16 * MC)

    # Drop the unused const-AP memsets bass emits unconditionally in its
    # preamble (the BIR verifier itself flags them as having no reader);
    # they serialize ~380ns on Pool ahead of the startup barrier.
    b0 = nc.m.functions[0].blocks[0]
    b0.instructions = [
        i
        for i in b0.instructions
        if not (
            type(i).__name__ == "InstMemset"
            and str(getattr(i.outs[0], "memref", "")).startswith("const-")
        )
    ]
    # Drop the exit all-engine-barrier semaphore ops: the SP stream already
    # ends on wait_ge(s_out) after the last output DMA receipt, so every
    # output byte is in HBM before any engine halts; the cross-engine
    # EVSEM handshake only aligns halt times (~260ns).
    for b in nc.m.functions[0].blocks:
        if str(getattr(b, "name", "")).endswith("_end"):
            b.instructions = [
                i
                for i in b.instructions
                if not (
                    type(i).__name__ == "InstEventSemaphore"
                    and str(i.name).startswith("aeb_barrier")
                )
            ]
    # Drop the startup all-engine barrier as well (~450ns): every
    # cross-engine dependency in this kernel is carried by its own
    # semaphores (s_init gates PE on Pool's memset; DMA sems gate all
    # consumers), and each engine's register preamble precedes its own
    # work within its own stream.
    b0.instructions = [
        i for i in b0.instructions if not str(i.name).startswith("barrier_")
    ]

    _cache[key] = nc
    return nc


def _pack_inputs(hidden_states, W_seq, hidden_embeddings, cp_weight):
    """Build the per-core packed SBUF images.

    xt image:   xt[c][p, k*RPC + n] = X[c*RPC + n, k*128 + p]
    W image:    w[p, k*R + r]       = W_seq[r, k*128 + p]
    h image:    h[r, j]             = hid_fac[j, r] = (hidden_embeddings * cp)[j, r]
    """
    ind = _np_in_dtype()
    X = hidden_states.reshape(ROWS, H)
    xt = (
        X.astype(ind)
        .reshape(N_CORES, RPC, KC, 128)  # [c, n, k, p]
        .transpose(0, 3, 2, 1)           # [c, p, k, n]
        .reshape(N_CORES, 128, KC * RPC)
    )
    w = (
        W_seq.astype(np.float32)
        .reshape(R, KC, 128)             # [r, k, p]
        .transpose(2, 1, 0)              # [p, k, r]
        .reshape(128, W_COLS)
        .astype(ind)
    )
    xw = np.ascontiguousarray(
        np.concatenate(
            [np.broadcast_to(w, (N_CORES, 128, W_COLS)), xt[:, :, :HALF]], axis=2
        )
    )                                    # [c, 128, W_COLS + HALF]
    xt23 = np.ascontiguousarray(xt[:, :, HALF:])
    h = np.ascontiguousarray(
        (hidden_embeddings * cp_weight[0][None, :]).T.astype(ind)
    )                                    # [64, 512]
    return xw, xt23, h


def _run_device(xw, xt23, h, trace=False, **run_kwargs):
    global LAST_RESULT
    from concourse.bass_utils import run_bass_kernel_spmd

    nc = _get_nc()
    in_maps = [{"xw": xw[c], "xt23": xt23[c], "h": h} for c in range(N_CORES)]
    res = run_bass_kernel_spmd(
        nc, in_maps, core_ids=list(range(N_CORES)), trace=trace, **run_kwargs
    )
    LAST_RESULT = res
    return np.concatenate([r["out"] for r in res.results], axis=0)  # [2048, 512]


def _host_reference(hidden_states, W_seq, hidden_embeddings, cp_weight):
    """Pure-numpy fallback (correct, host-only)."""
    hid_fac = hidden_embeddings * cp_weight[0][None, :]
    X = hidden_states.reshape(ROWS, H)
    return (X @ W_seq.T @ hid_fac.T).astype(np.float32)


def kernel(hidden_states, all_indices, W_seq, hidden_embeddings, cp_weight,
           trace=False, **run_kwargs):
    hidden_states = np.asarray(hidden_states, dtype=np.float32)
    W_seq = np.asarray(W_seq, dtype=np.float32)
    hidden_embeddings = np.asarray(hidden_embeddings, dtype=np.float32)
    cp_weight = np.asarray(cp_weight, dtype=np.float32)
    all_indices = np.asarray(all_indices)

    try:
        xw, xt23, h = _pack_inputs(
            hidden_states, W_seq, hidden_embeddings, cp_weight
        )
        Y = _run_device(xw, xt23, h, trace=trace, **run_kwargs)
    except Exception as e:  # device unavailable/wedged: stay correct on host
        import traceback

        traceback.print_exc()
        print(f"kernel: device path failed ({type(e).__name__}); "
              "falling back to host compute")
        Y = _host_reference(hidden_states, W_seq, hidden_embeddings, cp_weight)

    P = Y.reshape(B, S, H)

    n = all_indices.shape[0]
    si = all_indices[:, 0].astype(np.int64)
    hi = all_indices[:, 1].astype(np.int64)
    flat = si * H + hi
    if n == S * H and np.array_equal(flat, np.arange(S * H, dtype=np.int64)):
        return P  # cartesian-product indices: the gather is the identity
    return P.reshape(B, S * H)[:, flat].reshape(B, S, n // S)
